# revision 4
# baseline (speedup 1.0000x reference)
"""ConvAttention (GroupNorm + channel attention + residual) on 8 Trainium2
NeuronCores, data-parallel over the batch dimension (B=8 -> 1 item/core).

Per-core algorithm (x is (C=512, N=4096) for one batch item):
  1. GroupNorm(32 groups) via per-channel bn_stats + tiny indicator matmuls
     for the cross-partition group reduction / broadcast; the affine
     (x*a + b) runs on the Scalar engine, producing g in bf16.
  2. scores = q k^T (contracted over N) is computed via the Gram matrix:
         scores = Wq (g g^T) Wk^T + (Wq sg) (x) bk + bq (x) (Wk sg + N bk)
     with sg = row sums of g.  g g^T needs g^T, produced by PE transposes
     that are pipelined with the Gram accumulation.
  3. softmax rows, fused: exp((s - max) * alpha) on the Scalar engine with
     accumulated row sums; probs scaled by 1/sum, then PE-transposed.
  4. attn^T = g^T M^T + 1 (x) (bv^T probs^T), with M^T = Wv^T probs^T.
     attn^T lands in (N, C) layout, which makes the reference's
     permute+reshape a flat-memory no-op:  out.flat = x.flat + attnT.flat.
  5. Residual x is re-streamed into the (N, C) tile layout by DMA and the
     final add is fused into the PSUM evacuation.
"""
import sys

if "/opt/trn_rl_repo" not in sys.path:
    sys.path.insert(0, "/opt/trn_rl_repo")

from contextlib import ExitStack

import ml_dtypes
import numpy as np

import concourse.bass as bass
import concourse.tile as tile
from concourse import bacc, mybir
from concourse import bass_utils
from concourse.masks import make_identity

BF16 = ml_dtypes.bfloat16
bf = mybir.dt.bfloat16
f32 = mybir.dt.float32

B, C, H, W = 8, 512, 64, 64
N = H * W            # 4096 spatial tokens
GROUPS = 32
GS = C // GROUPS     # 16 channels per group
EPS = 1e-6
ALPHA = float(C) ** -0.5
P = 128
CT = C // P          # 4 channel tiles
NT = N // P          # 32 spatial tiles
SUB = 512            # bn_stats subgroup width
NSUB = N // SUB      # 8

AF = mybir.ActivationFunctionType
AX = mybir.AxisListType
OP = mybir.AluOpType


def _build_program():
    nc = bacc.Bacc("TRN2", target_bir_lowering=False, debug=False, num_devices=B)

    x_d = nc.dram_tensor("x", (C, N), f32, kind="ExternalInput").ap()
    wqT_d = nc.dram_tensor("wqT", (C, C), bf, kind="ExternalInput").ap()
    wkT_d = nc.dram_tensor("wkT", (C, C), bf, kind="ExternalInput").ap()
    wv_d = nc.dram_tensor("wv", (C, C), bf, kind="ExternalInput").ap()
    bqr_d = nc.dram_tensor("bq_row", (1, C), bf, kind="ExternalInput").ap()
    bkr_d = nc.dram_tensor("bk_row", (1, C), bf, kind="ExternalInput").ap()
    bk4_d = nc.dram_tensor("bk_n", (1, C), f32, kind="ExternalInput").ap()
    bv_d = nc.dram_tensor("bv_col", (C, 1), bf, kind="ExternalInput").ap()
    gnw_d = nc.dram_tensor("gnw", (C, 1), f32, kind="ExternalInput").ap()
    gnb_d = nc.dram_tensor("gnb", (C, 1), f32, kind="ExternalInput").ap()
    i16_d = nc.dram_tensor("ind16", (C, GROUPS), f32, kind="ExternalInput").ap()
    iT_d = nc.dram_tensor("indT01", (GROUPS, C), f32, kind="ExternalInput").ap()
    out_d = nc.dram_tensor("out", (N, C), f32, kind="ExternalOutput").ap()

    with tile.TileContext(nc) as tc, ExitStack() as ctx:
        consts = ctx.enter_context(tc.tile_pool(name="consts", bufs=1))
        px = ctx.enter_context(tc.tile_pool(name="px", bufs=1))
        pg = ctx.enter_context(tc.tile_pool(name="pg", bufs=1))
        pmats = ctx.enter_context(tc.tile_pool(name="pmats", bufs=1))
        pgt = ctx.enter_context(tc.tile_pool(name="pgt", bufs=3))
        psmall = ctx.enter_context(tc.tile_pool(name="psmall", bufs=4))
        presid = ctx.enter_context(tc.tile_pool(name="presid", bufs=6))
        pout = ctx.enter_context(tc.tile_pool(name="pout", bufs=6))
        # PSUM: 4 (gram accum) + 2 (transpose) + 2 (rotating matmul out) = 8 banks
        ps_acc = ctx.enter_context(tc.tile_pool(name="ps_acc", bufs=1, space="PSUM"))
        ps_tr = ctx.enter_context(tc.tile_pool(name="ps_tr", bufs=2, space="PSUM"))
        ps_big = ctx.enter_context(tc.tile_pool(name="ps_big", bufs=2, space="PSUM"))

        # ---------------- constants / weights ----------------
        ident = consts.tile([P, P], bf, tag="ident")
        make_identity(nc, ident)

        wq_sb, wk_sb, wv_sb = [], [], []
        for t in range(CT):
            wq_t = consts.tile([P, C], bf, tag=f"wq{t}")
            nc.sync.dma_start(wq_t, wqT_d[t * P:(t + 1) * P, :])
            wq_sb.append(wq_t)
            wk_t = consts.tile([P, C], bf, tag=f"wk{t}")
            nc.sync.dma_start(wk_t, wkT_d[t * P:(t + 1) * P, :])
            wk_sb.append(wk_t)
            wv_t = consts.tile([P, C], bf, tag=f"wv{t}")
            nc.sync.dma_start(wv_t, wv_d[t * P:(t + 1) * P, :])
            wv_sb.append(wv_t)

        bqr = consts.tile([1, C], bf, tag="bqr")
        nc.sync.dma_start(bqr, bqr_d)
        bkr = consts.tile([1, C], bf, tag="bkr")
        nc.sync.dma_start(bkr, bkr_d)
        bk4 = consts.tile([1, C], f32, tag="bk4")
        nc.sync.dma_start(bk4, bk4_d)

        bv_sb, gnw_sb, gnb_sb, i16_sb = [], [], [], []
        for t in range(CT):
            bv_t = consts.tile([P, 1], bf, tag=f"bv{t}")
            nc.sync.dma_start(bv_t, bv_d[t * P:(t + 1) * P, :])
            bv_sb.append(bv_t)
            gnw_t = consts.tile([P, 1], f32, tag=f"gnw{t}")
            nc.sync.dma_start(gnw_t, gnw_d[t * P:(t + 1) * P, :])
            gnw_sb.append(gnw_t)
            gnb_t = consts.tile([P, 1], f32, tag=f"gnb{t}")
            nc.sync.dma_start(gnb_t, gnb_d[t * P:(t + 1) * P, :])
            gnb_sb.append(gnb_t)
            i16_t = consts.tile([P, GROUPS], f32, tag=f"i16{t}")
            nc.sync.dma_start(i16_t, i16_d[t * P:(t + 1) * P, :])
            i16_sb.append(i16_t)
        iT_sb = consts.tile([GROUPS, C], f32, tag="iT")
        nc.sync.dma_start(iT_sb, iT_d)
        eps32 = consts.tile([GROUPS, 1], f32, tag="eps32")
        nc.vector.memset(eps32, EPS)
        ones1 = consts.tile([1, P], bf, tag="ones1")
        nc.vector.memset(ones1, 1.0)

        # ---------------- phase 1: load x, per-channel stats ----------------
        x_sb, mv_sb = [], []
        # group stats accumulator: [32 groups, (mean_of_means, mean_of_E2)]
        gstats = ps_acc.tile([GROUPS, 2], f32, tag="G0")
        for ci in range(CT):
            x_t = px.tile([P, N], f32, tag=f"x{ci}")
            nc.sync.dma_start(x_t, x_d[ci * P:(ci + 1) * P, :])
            x_sb.append(x_t)

            stats = psmall.tile([P, NSUB, 6], f32, tag="stats")
            xv = x_t.rearrange("p (s f) -> p s f", f=SUB)
            for s in range(NSUB):
                nc.vector.bn_stats(out=stats[:, s, :], in_=xv[:, s, :])
            mv = psmall.tile([P, 2], f32, tag=f"mv{ci}", bufs=1)
            nc.vector.bn_aggr(out=mv, in_=stats)
            mv_sb.append(mv)

            # st2 = [mu_p, E[x^2]_p] per channel
            st2 = psmall.tile([P, 2], f32, tag="st2")
            nc.vector.tensor_copy(st2[:, 0:1], mv[:, 0:1])
            e2 = psmall.tile([P, 1], f32, tag="e2")
            nc.vector.tensor_scalar(e2, mv[:, 0:1], mv[:, 0:1], None, op0=OP.mult)
            nc.vector.tensor_tensor(st2[:, 1:2], e2, mv[:, 1:2], OP.add)
            # accumulate group means (1/16-weighted indicator)
            nc.tensor.matmul(gstats, lhsT=i16_sb[ci], rhs=st2,
                             start=(ci == 0), stop=(ci == CT - 1))

        # group variance -> rstd
        gtmp = psmall.tile([GROUPS, 1], f32, tag="gtmp")
        nc.vector.tensor_scalar(gtmp, gstats[:, 0:1], gstats[:, 0:1], None, op0=OP.mult)
        gvar = psmall.tile([GROUPS, 1], f32, tag="gvar")
        nc.vector.tensor_tensor(gvar, gstats[:, 1:2], gtmp, OP.subtract)
        gsd = psmall.tile([GROUPS, 1], f32, tag="gsd")
        nc.scalar.activation(gsd, gvar, AF.Sqrt, bias=eps32, scale=1.0)
        grs = psmall.tile([GROUPS, 1], f32, tag="grs")
        nc.vector.reciprocal(grs, gsd)
        gr2 = psmall.tile([GROUPS, 2], f32, tag="gr2")
        nc.vector.tensor_copy(gr2[:, 0:1], gstats[:, 0:1])
        nc.vector.tensor_copy(gr2[:, 1:2], grs)

        # ---------------- phase 2: per-channel affine -> g (bf16), sg ----------------
        g_sb, sg_sb = [], []
        for ci in range(CT):
            bc = ps_big.tile([P, 2], f32, tag="big")
            nc.tensor.matmul(bc, lhsT=iT_sb[:, ci * P:(ci + 1) * P], rhs=gr2,
                             start=True, stop=True)
            a_col = psmall.tile([P, 1], f32, tag=f"a{ci}", bufs=1)
            nc.vector.tensor_tensor(a_col, gnw_sb[ci], bc[:, 1:2], OP.mult)
            tmp = psmall.tile([P, 1], f32, tag="tmp")
            nc.vector.tensor_tensor(tmp, bc[:, 0:1], a_col, OP.mult)
            b_col = psmall.tile([P, 1], f32, tag=f"b{ci}", bufs=1)
            nc.vector.tensor_tensor(b_col, gnb_sb[ci], tmp, OP.subtract)

            g_t = pg.tile([P, N], bf, tag=f"g{ci}")
            nc.scalar.activation(g_t, x_sb[ci], AF.Identity, bias=b_col, scale=a_col)
            g_sb.append(g_t)

            # sg = N * (a * mu_p + b)  (row sums of g), as bf16 column
            t2 = psmall.tile([P, 1], f32, tag="t2")
            nc.vector.tensor_tensor(t2, a_col, mv_sb[ci][:, 0:1], OP.mult)
            nc.vector.tensor_tensor(t2, t2, b_col, OP.add)
            sg_t = consts.tile([P, 1], bf, tag=f"sg{ci}")
            nc.vector.tensor_scalar(sg_t, t2, float(N), None, op0=OP.mult)
            sg_sb.append(sg_t)

        # ---------------- phase 3: Gram = g g^T, pipelined with g^T transposes ----------------
        G_ps = [ps_acc.tile([P, C], f32, tag=f"G{i}", name=f"Gps{i}")
                for i in range(CT)]
        # NOTE: tag G0 reused after gstats is fully consumed above.
        prev_gt = None
        for nt in range(NT + 1):
            if nt < NT:
                trp = ps_tr.tile([P, C], bf, tag="tr")
                for it in range(CT):
                    nc.tensor.transpose(trp[:, it * P:(it + 1) * P],
                                        g_sb[it][:, nt * P:(nt + 1) * P], ident)
                gt = pgt.tile([P, C], bf, tag="gt")
                nc.vector.tensor_copy(gt, trp)
            else:
                gt = None
            if prev_gt is not None:
                for io in range(CT):
                    nc.tensor.matmul(G_ps[io], lhsT=prev_gt[:, io * P:(io + 1) * P],
                                     rhs=prev_gt, start=(nt == 1), stop=(nt == NT))
            prev_gt = gt

        G_sb = []
        for io in range(CT):
            G_t = pmats.tile([P, C], bf, tag=f"Gm{io}")
            nc.vector.tensor_copy(G_t, G_ps[io])
            G_sb.append(G_t)

        # ---------------- phase 4: bias rows t1 = Wq sg, u = Wk sg + N bk ----------------
        t1p = ps_big.tile([1, C], f32, tag="big")
        for it in range(CT):
            nc.tensor.matmul(t1p, lhsT=sg_sb[it], rhs=wq_sb[it],
                             start=(it == 0), stop=(it == CT - 1))
        t2p = ps_big.tile([1, C], f32, tag="big")
        for it in range(CT):
            nc.tensor.matmul(t2p, lhsT=sg_sb[it], rhs=wk_sb[it],
                             start=(it == 0), stop=(it == CT - 1))
        t1row = consts.tile([1, C], bf, tag="t1row")
        nc.vector.tensor_copy(t1row, t1p)
        urow = consts.tile([1, C], bf, tag="urow")
        nc.vector.tensor_tensor(urow, t2p, bk4, OP.add)

        # ---------------- phase 5: A = Wq G, A^T ----------------
        A_sb = []
        for ct in range(CT):
            Ap = ps_big.tile([P, C], f32, tag="big")
            for it in range(CT):
                nc.tensor.matmul(Ap, lhsT=wq_sb[it][:, ct * P:(ct + 1) * P],
                                 rhs=G_sb[it], start=(it == 0), stop=(it == CT - 1))
            A_t = pmats.tile([P, C], bf, tag=f"A{ct}")
            nc.vector.tensor_copy(A_t, Ap)
            A_sb.append(A_t)

        AT_sb = [pmats.tile([P, C], bf, tag=f"AT{jt}", name=f"ATsb{jt}")
                 for jt in range(CT)]
        for ct in range(CT):
            trp = ps_tr.tile([P, C], bf, tag="tr")
            for jt in range(CT):
                nc.tensor.transpose(trp[:, jt * P:(jt + 1) * P],
                                    A_sb[ct][:, jt * P:(jt + 1) * P], ident)
            for jt in range(CT):
                nc.vector.tensor_copy(AT_sb[jt][:, ct * P:(ct + 1) * P],
                                      trp[:, jt * P:(jt + 1) * P])

        # ---------------- phase 6: scores + softmax ----------------
        pr_sb = []
        for ct in range(CT):
            scp = ps_big.tile([P, C], f32, tag="big")
            for jt in range(CT):
                nc.tensor.matmul(scp, lhsT=AT_sb[jt][:, ct * P:(ct + 1) * P],
                                 rhs=wk_sb[jt], start=(jt == 0), stop=False)
            nc.tensor.matmul(scp, lhsT=t1row[0:1, ct * P:(ct + 1) * P], rhs=bkr,
                             start=False, stop=False)
            nc.tensor.matmul(scp, lhsT=bqr[0:1, ct * P:(ct + 1) * P], rhs=urow,
                             start=False, stop=True)
            nm = psmall.tile([P, 1], f32, tag="nm")
            nc.vector.reduce_max(nm, scp, axis=AX.X, negate=True)
            nma = psmall.tile([P, 1], f32, tag="nma")
            nc.vector.tensor_scalar(nma, nm, ALPHA, None, op0=OP.mult)
            se = psmall.tile([P, 1], f32, tag="se")
            pr_t = pmats.tile([P, C], bf, tag=f"pr{ct}")
            nc.scalar.activation(pr_t, scp, AF.Exp, bias=nma, scale=ALPHA,
                                 accum_out=se)
            ri = psmall.tile([P, 1], f32, tag="ri")
            nc.vector.reciprocal(ri, se)
            nc.vector.tensor_scalar_mul(pr_t, pr_t, ri)
            pr_sb.append(pr_t)

        # probs^T
        prT_sb = [pmats.tile([P, C], bf, tag=f"prT{dt}", name=f"prTsb{dt}")
                  for dt in range(CT)]
        for ct in range(CT):
            trp = ps_tr.tile([P, C], bf, tag="tr")
            for dt in range(CT):
                nc.tensor.transpose(trp[:, dt * P:(dt + 1) * P],
                                    pr_sb[ct][:, dt * P:(dt + 1) * P], ident)
            for dt in range(CT):
                nc.vector.tensor_copy(prT_sb[dt][:, ct * P:(ct + 1) * P],
                                      trp[:, dt * P:(dt + 1) * P])

        # ---------------- phase 7: M^T = Wv^T probs^T, pv row ----------------
        MT_sb = []
        for it in range(CT):
            Mp = ps_big.tile([P, C], f32, tag="big")
            for dt in range(CT):
                nc.tensor.matmul(Mp, lhsT=wv_sb[dt][:, it * P:(it + 1) * P],
                                 rhs=prT_sb[dt], start=(dt == 0), stop=(dt == CT - 1))
            MT_t = pmats.tile([P, C], bf, tag=f"MT{it}")
            nc.vector.tensor_copy(MT_t, Mp)
            MT_sb.append(MT_t)

        pvp = ps_big.tile([1, C], f32, tag="big")
        for dt in range(CT):
            nc.tensor.matmul(pvp, lhsT=bv_sb[dt], rhs=prT_sb[dt],
                             start=(dt == 0), stop=(dt == CT - 1))
        pvrow = consts.tile([1, C], bf, tag="pvrow")
        nc.vector.tensor_copy(pvrow, pvp)

        # ---------------- phase 8: attn^T + residual + store ----------------
        x_v = [x_sb[ci].rearrange("p (u f) -> p u f", u=N // SUB) for ci in range(CT)]
        for nt in range(NT):
            at = ps_big.tile([P, C], f32, tag="big")
            for it in range(CT):
                nc.tensor.matmul(at, lhsT=g_sb[it][:, nt * P:(nt + 1) * P],
                                 rhs=MT_sb[it], start=(it == 0), stop=False)
            nc.tensor.matmul(at, lhsT=ones1, rhs=pvrow, start=False, stop=True)

            resid = presid.tile([P, C], f32, tag="resid")
            ci, lo = nt // 8, nt % 8
            nc.sync.dma_start(
                resid,
                x_sb[ci][16 * lo:16 * (lo + 1), :].rearrange(
                    "p (u f) -> p u f", u=8),
            )
            osb = pout.tile([P, C], f32, tag="o")
            nc.vector.tensor_tensor(osb, at, resid, OP.add)
            nc.sync.dma_start(out_d[nt * P:(nt + 1) * P, :], osb)

    nc.compile()
    return nc


_NC = None


def _get_program():
    global _NC
    if _NC is None:
        _NC = _build_program()
    return _NC


def _stage_inputs(x, gn_w, gn_b, wq, bq, wk, bk, wv, bv):
    """Build the per-core input maps (host-side sharding / layout prep)."""
    x = np.asarray(x, dtype=np.float32).reshape(B, C, N)
    shared = {
        "wqT": np.ascontiguousarray(np.asarray(wq, np.float32).T).astype(BF16),
        "wkT": np.ascontiguousarray(np.asarray(wk, np.float32).T).astype(BF16),
        "wv": np.ascontiguousarray(np.asarray(wv, np.float32)).astype(BF16),
        "bq_row": np.asarray(bq, np.float32).reshape(1, C).astype(BF16),
        "bk_row": np.asarray(bk, np.float32).reshape(1, C).astype(BF16),
        "bk_n": (float(N) * np.asarray(bk, np.float32)).reshape(1, C),
        "bv_col": np.asarray(bv, np.float32).reshape(C, 1).astype(BF16),
        "gnw": np.asarray(gn_w, np.float32).reshape(C, 1),
        "gnb": np.asarray(gn_b, np.float32).reshape(C, 1),
    }
    ind16 = np.zeros((C, GROUPS), np.float32)
    indT = np.zeros((GROUPS, C), np.float32)
    for c in range(C):
        ind16[c, c // GS] = 1.0 / GS
        indT[c // GS, c] = 1.0
    shared["ind16"] = ind16
    shared["indT01"] = indT

    in_maps = []
    for b in range(B):
        m = dict(shared)
        m["x"] = np.ascontiguousarray(x[b])
        in_maps.append(m)
    return in_maps


def kernel(x, gn_w, gn_b, wq, bq, wk, bk, wv, bv, _trace=False, _tmpdir=None):
    nc = _get_program()
    in_maps = _stage_inputs(x, gn_w, gn_b, wq, bq, wk, bk, wv, bv)
    res = bass_utils.run_bass_kernel_spmd(
        nc, in_maps, core_ids=list(range(B)), trace=_trace, tmpdir=_tmpdir,
    )
    out = np.stack([res.results[b]["out"].reshape(C, H, W) for b in range(B)])
    if _trace:
        kernel._last_results = res
    return out.astype(np.float32)


# revision 8
# speedup vs baseline: 1.1090x; 1.1090x over previous
"""ConvAttention (GroupNorm + channel attention + residual) on 8 Trainium2
NeuronCores, data-parallel over the batch dimension (B=8 -> 1 item/core).

Per-core algorithm (x is (C=512, N=4096) for one batch item):
  1. GroupNorm(32 groups) via per-channel bn_stats + tiny indicator matmuls
     for the cross-partition group reduction / broadcast; the affine
     (x*a + b) runs on the Scalar engine, producing g in bf16.
  2. scores = q k^T (contracted over N) is computed via the Gram matrix:
         scores = Wq (g g^T) Wk^T + (Wq sg) (x) bk + bq (x) (Wk sg + N bk)
     with sg = row sums of g.  g g^T needs g^T, produced by PE transposes
     that are pipelined with the Gram accumulation.
  3. softmax rows, fused: exp((s - max) * alpha) on the Scalar engine with
     accumulated row sums; probs scaled by 1/sum, then PE-transposed.
  4. attn^T = g^T M^T + 1 (x) (bv^T probs^T), with M^T = Wv^T probs^T.
     attn^T lands in (N, C) layout, which makes the reference's
     permute+reshape a flat-memory no-op:  out.flat = x.flat + attnT.flat.
  5. Residual x is re-streamed into the (N, C) tile layout by DMA and the
     final add is fused into the PSUM evacuation.
"""
import sys

if "/opt/trn_rl_repo" not in sys.path:
    sys.path.insert(0, "/opt/trn_rl_repo")

from contextlib import ExitStack

import ml_dtypes
import numpy as np

import concourse.bass as bass
import concourse.tile as tile
from concourse import bacc, mybir
from concourse import bass_utils
from concourse.masks import make_identity

BF16 = ml_dtypes.bfloat16
bf = mybir.dt.bfloat16
f32 = mybir.dt.float32

B, C, H, W = 8, 512, 64, 64
N = H * W            # 4096 spatial tokens
GROUPS = 32
GS = C // GROUPS     # 16 channels per group
EPS = 1e-6
ALPHA = float(C) ** -0.5
P = 128
CT = C // P          # 4 channel tiles
NT = N // P          # 32 spatial tiles
SUB = 512            # bn_stats subgroup width
NSUB = N // SUB      # 8

AF = mybir.ActivationFunctionType
AX = mybir.AxisListType
OP = mybir.AluOpType


def _build_program():
    nc = bacc.Bacc("TRN2", target_bir_lowering=False, debug=False, num_devices=B)

    x_d = nc.dram_tensor("x", (C, N), f32, kind="ExternalInput").ap()
    wqT_d = nc.dram_tensor("wqT", (C, C), bf, kind="ExternalInput").ap()
    wkT_d = nc.dram_tensor("wkT", (C, C), bf, kind="ExternalInput").ap()
    wv_d = nc.dram_tensor("wv", (C, C), bf, kind="ExternalInput").ap()
    bqr_d = nc.dram_tensor("bq_row", (1, C), bf, kind="ExternalInput").ap()
    bkr_d = nc.dram_tensor("bk_row", (1, C), bf, kind="ExternalInput").ap()
    bk4_d = nc.dram_tensor("bk_n", (1, C), f32, kind="ExternalInput").ap()
    bv_d = nc.dram_tensor("bv_col", (C, 1), bf, kind="ExternalInput").ap()
    gnw_d = nc.dram_tensor("gnw", (C, 1), f32, kind="ExternalInput").ap()
    gnb_d = nc.dram_tensor("gnb", (C, 1), f32, kind="ExternalInput").ap()
    i16_d = nc.dram_tensor("ind16", (C, 8), f32, kind="ExternalInput").ap()
    iT_d = nc.dram_tensor("indT01", (8, P), f32, kind="ExternalInput").ap()
    out_d = nc.dram_tensor("out", (N, C), f32, kind="ExternalOutput").ap()

    with tile.TileContext(nc) as tc, ExitStack() as ctx:
        consts = ctx.enter_context(tc.tile_pool(name="consts", bufs=1))
        px = ctx.enter_context(tc.tile_pool(name="px", bufs=1))
        pg = ctx.enter_context(tc.tile_pool(name="pg", bufs=1))
        pmats = ctx.enter_context(tc.tile_pool(name="pmats", bufs=1))
        pgt = ctx.enter_context(tc.tile_pool(name="pgt", bufs=3))
        psmall = ctx.enter_context(tc.tile_pool(name="psmall", bufs=4))
        presid = ctx.enter_context(tc.tile_pool(name="presid", bufs=6))
        pout = ctx.enter_context(tc.tile_pool(name="pout", bufs=6))
        # PSUM: 4 (gram accum) + 2 (transpose) + 2 (rotating matmul out) = 8 banks
        ps_acc = ctx.enter_context(tc.tile_pool(name="ps_acc", bufs=1, space="PSUM"))
        ps_tr = ctx.enter_context(tc.tile_pool(name="ps_tr", bufs=2, space="PSUM"))
        ps_big = ctx.enter_context(tc.tile_pool(name="ps_big", bufs=2, space="PSUM"))

        # ---------------- constants / weights ----------------
        ident = consts.tile([P, P], bf, tag="ident")
        make_identity(nc, ident)

        wq_sb, wk_sb, wv_sb = [], [], []
        for t in range(CT):
            wq_t = consts.tile([P, C], bf, tag=f"wq{t}")
            nc.sync.dma_start(wq_t, wqT_d[t * P:(t + 1) * P, :])
            wq_sb.append(wq_t)
            wk_t = consts.tile([P, C], bf, tag=f"wk{t}")
            nc.sync.dma_start(wk_t, wkT_d[t * P:(t + 1) * P, :])
            wk_sb.append(wk_t)
            wv_t = consts.tile([P, C], bf, tag=f"wv{t}")
            nc.sync.dma_start(wv_t, wv_d[t * P:(t + 1) * P, :])
            wv_sb.append(wv_t)

        bqr = consts.tile([1, C], bf, tag="bqr")
        nc.sync.dma_start(bqr, bqr_d)
        bkr = consts.tile([1, C], bf, tag="bkr")
        nc.sync.dma_start(bkr, bkr_d)
        bk4 = consts.tile([1, C], f32, tag="bk4")
        nc.sync.dma_start(bk4, bk4_d)

        bv_sb, gnw_sb, gnb_sb, i16_sb = [], [], [], []
        for t in range(CT):
            bv_t = consts.tile([P, 1], bf, tag=f"bv{t}")
            nc.sync.dma_start(bv_t, bv_d[t * P:(t + 1) * P, :])
            bv_sb.append(bv_t)
            gnw_t = consts.tile([P, 1], f32, tag=f"gnw{t}")
            nc.sync.dma_start(gnw_t, gnw_d[t * P:(t + 1) * P, :])
            gnw_sb.append(gnw_t)
            gnb_t = consts.tile([P, 1], f32, tag=f"gnb{t}")
            nc.sync.dma_start(gnb_t, gnb_d[t * P:(t + 1) * P, :])
            gnb_sb.append(gnb_t)
            i16_t = consts.tile([P, 8], f32, tag=f"i16{t}")
            nc.sync.dma_start(i16_t, i16_d[t * P:(t + 1) * P, :])
            i16_sb.append(i16_t)
        iT_sb = consts.tile([8, P], f32, tag="iT")
        nc.sync.dma_start(iT_sb, iT_d)
        eps8 = consts.tile([8, 1], f32, tag="eps8")
        nc.vector.memset(eps8, EPS)
        ones1 = consts.tile([1, P], bf, tag="ones1")
        nc.vector.memset(ones1, 1.0)

        # ---------------- phase 1+2: per-tile GroupNorm (fully pipelined) ------
        # Groups (16ch) never cross a 128-channel tile, so each tile reduces its
        # own 8 groups: stats -> local indicator matmuls -> affine, no barrier.
        dmae = [nc.sync, nc.scalar]  # alternate the two HWDGE queues
        x_sb, g_sb, sg_sb = [], [], []
        for ci in range(CT):
            x_t = px.tile([P, N], f32, tag=f"x{ci}")
            nc.sync.dma_start(x_t[:, :N // 2], x_d[ci * P:(ci + 1) * P, :N // 2])
            nc.scalar.dma_start(x_t[:, N // 2:], x_d[ci * P:(ci + 1) * P, N // 2:])
            x_sb.append(x_t)

            stats = psmall.tile([P, NSUB, 6], f32, tag="stats")
            xv = x_t.rearrange("p (s f) -> p s f", f=SUB)
            for s in range(NSUB):
                nc.vector.bn_stats(out=stats[:, s, :], in_=xv[:, s, :])
            mv = psmall.tile([P, 2], f32, tag=f"mv{ci}", bufs=1)
            nc.vector.bn_aggr(out=mv, in_=stats)

            # st2 = [mu_p, E[x^2]_p] per channel
            st2 = psmall.tile([P, 2], f32, tag="st2")
            nc.vector.tensor_copy(st2[:, 0:1], mv[:, 0:1])
            e2 = psmall.tile([P, 1], f32, tag="e2")
            nc.vector.tensor_scalar(e2, mv[:, 0:1], mv[:, 0:1], None, op0=OP.mult)
            nc.vector.tensor_tensor(st2[:, 1:2], e2, mv[:, 1:2], OP.add)
            # local 8-group reduction (1/16-weighted indicator)
            gst = ps_big.tile([8, 2], f32, tag="big")
            nc.tensor.matmul(gst, lhsT=i16_sb[ci], rhs=st2, start=True, stop=True)

            gtmp = psmall.tile([8, 1], f32, tag="gtmp")
            nc.vector.tensor_scalar(gtmp, gst[:, 0:1], gst[:, 0:1], None, op0=OP.mult)
            gvar = psmall.tile([8, 1], f32, tag="gvar")
            nc.vector.tensor_tensor(gvar, gst[:, 1:2], gtmp, OP.subtract)
            gsd = psmall.tile([8, 1], f32, tag="gsd")
            nc.scalar.activation(gsd, gvar, AF.Sqrt, bias=eps8, scale=1.0)
            grs = psmall.tile([8, 1], f32, tag="grs")
            nc.vector.reciprocal(grs, gsd)
            gr2 = psmall.tile([8, 2], f32, tag="gr2")
            nc.vector.tensor_copy(gr2[:, 0:1], gst[:, 0:1])
            nc.vector.tensor_copy(gr2[:, 1:2], grs)

            bc = ps_big.tile([P, 2], f32, tag="big")
            nc.tensor.matmul(bc, lhsT=iT_sb, rhs=gr2, start=True, stop=True)
            a_col = psmall.tile([P, 1], f32, tag=f"a{ci}", bufs=1)
            nc.vector.tensor_tensor(a_col, gnw_sb[ci], bc[:, 1:2], OP.mult)
            tmp = psmall.tile([P, 1], f32, tag="tmp")
            nc.vector.tensor_tensor(tmp, bc[:, 0:1], a_col, OP.mult)
            b_col = psmall.tile([P, 1], f32, tag=f"b{ci}", bufs=1)
            nc.vector.tensor_tensor(b_col, gnb_sb[ci], tmp, OP.subtract)

            g_t = pg.tile([P, N], bf, tag=f"g{ci}")
            if ci % 2 == 0:
                # Scalar engine: g = Identity(x * a + b)
                nc.scalar.activation(g_t, x_sb[ci], AF.Identity,
                                     bias=b_col, scale=a_col)
            else:
                # Vector engine: same affine, keeps both engines busy
                nc.vector.tensor_scalar(g_t, x_sb[ci], a_col, b_col,
                                        op0=OP.mult, op1=OP.add)
            g_sb.append(g_t)

            # sg = N * (a * mu_p + b)  (row sums of g), as bf16 column
            t2 = psmall.tile([P, 1], f32, tag="t2")
            nc.vector.tensor_tensor(t2, a_col, mv[:, 0:1], OP.mult)
            nc.vector.tensor_tensor(t2, t2, b_col, OP.add)
            sg_t = consts.tile([P, 1], bf, tag=f"sg{ci}")
            nc.vector.tensor_scalar(sg_t, t2, float(N), None, op0=OP.mult)
            sg_sb.append(sg_t)

        # ---------------- phase 3: Gram = g g^T, pipelined with g^T transposes ----------------
        G_ps = [ps_acc.tile([P, C], f32, tag=f"G{i}", name=f"Gps{i}")
                for i in range(CT)]
        # NOTE: tag G0 reused after gstats is fully consumed above.
        prev_gt = None
        for nt in range(NT + 1):
            if nt < NT:
                trp = ps_tr.tile([P, C], bf, tag="tr")
                for it in range(CT):
                    nc.tensor.transpose(trp[:, it * P:(it + 1) * P],
                                        g_sb[it][:, nt * P:(nt + 1) * P], ident)
                gt = pgt.tile([P, C], bf, tag="gt")
                nc.vector.tensor_copy(gt, trp)
            else:
                gt = None
            if prev_gt is not None:
                for io in range(CT):
                    nc.tensor.matmul(G_ps[io], lhsT=prev_gt[:, io * P:(io + 1) * P],
                                     rhs=prev_gt, start=(nt == 1), stop=(nt == NT))
            prev_gt = gt

        G_sb = []
        for io in range(CT):
            G_t = pmats.tile([P, C], bf, tag=f"Gm{io}")
            nc.vector.tensor_copy(G_t, G_ps[io])
            G_sb.append(G_t)

        # ---------------- phase 4: bias rows t1 = Wq sg, u = Wk sg + N bk ----------------
        t1p = ps_big.tile([1, C], f32, tag="big")
        for it in range(CT):
            nc.tensor.matmul(t1p, lhsT=sg_sb[it], rhs=wq_sb[it],
                             start=(it == 0), stop=(it == CT - 1))
        t2p = ps_big.tile([1, C], f32, tag="big")
        for it in range(CT):
            nc.tensor.matmul(t2p, lhsT=sg_sb[it], rhs=wk_sb[it],
                             start=(it == 0), stop=(it == CT - 1))
        t1row = consts.tile([1, C], bf, tag="t1row")
        nc.vector.tensor_copy(t1row, t1p)
        urow = consts.tile([1, C], bf, tag="urow")
        nc.vector.tensor_tensor(urow, t2p, bk4, OP.add)

        # ---------------- phase 5: A = Wq G, A^T ----------------
        A_sb = []
        for ct in range(CT):
            Ap = ps_big.tile([P, C], f32, tag="big")
            for it in range(CT):
                nc.tensor.matmul(Ap, lhsT=wq_sb[it][:, ct * P:(ct + 1) * P],
                                 rhs=G_sb[it], start=(it == 0), stop=(it == CT - 1))
            A_t = pmats.tile([P, C], bf, tag=f"A{ct}")
            nc.vector.tensor_copy(A_t, Ap)
            A_sb.append(A_t)

        AT_sb = [pmats.tile([P, C], bf, tag=f"AT{jt}", name=f"ATsb{jt}")
                 for jt in range(CT)]
        for ct in range(CT):
            trp = ps_tr.tile([P, C], bf, tag="tr")
            for jt in range(CT):
                nc.tensor.transpose(trp[:, jt * P:(jt + 1) * P],
                                    A_sb[ct][:, jt * P:(jt + 1) * P], ident)
            for jt in range(CT):
                nc.vector.tensor_copy(AT_sb[jt][:, ct * P:(ct + 1) * P],
                                      trp[:, jt * P:(jt + 1) * P])

        # ---------------- phase 6: scores + softmax ----------------
        pr_sb = []
        for ct in range(CT):
            scp = ps_big.tile([P, C], f32, tag="big")
            for jt in range(CT):
                nc.tensor.matmul(scp, lhsT=AT_sb[jt][:, ct * P:(ct + 1) * P],
                                 rhs=wk_sb[jt], start=(jt == 0), stop=False)
            nc.tensor.matmul(scp, lhsT=t1row[0:1, ct * P:(ct + 1) * P], rhs=bkr,
                             start=False, stop=False)
            nc.tensor.matmul(scp, lhsT=bqr[0:1, ct * P:(ct + 1) * P], rhs=urow,
                             start=False, stop=True)
            nm = psmall.tile([P, 1], f32, tag="nm")
            nc.vector.reduce_max(nm, scp, axis=AX.X, negate=True)
            nma = psmall.tile([P, 1], f32, tag="nma")
            nc.vector.tensor_scalar(nma, nm, ALPHA, None, op0=OP.mult)
            se = psmall.tile([P, 1], f32, tag="se")
            pr_t = pmats.tile([P, C], bf, tag=f"pr{ct}")
            nc.scalar.activation(pr_t, scp, AF.Exp, bias=nma, scale=ALPHA,
                                 accum_out=se)
            ri = psmall.tile([P, 1], f32, tag="ri")
            nc.vector.reciprocal(ri, se)
            nc.vector.tensor_scalar_mul(pr_t, pr_t, ri)
            pr_sb.append(pr_t)

        # probs^T
        prT_sb = [pmats.tile([P, C], bf, tag=f"prT{dt}", name=f"prTsb{dt}")
                  for dt in range(CT)]
        for ct in range(CT):
            trp = ps_tr.tile([P, C], bf, tag="tr")
            for dt in range(CT):
                nc.tensor.transpose(trp[:, dt * P:(dt + 1) * P],
                                    pr_sb[ct][:, dt * P:(dt + 1) * P], ident)
            for dt in range(CT):
                nc.vector.tensor_copy(prT_sb[dt][:, ct * P:(ct + 1) * P],
                                      trp[:, dt * P:(dt + 1) * P])

        # ---------------- phase 7: M^T = Wv^T probs^T, pv row ----------------
        MT_sb = []
        for it in range(CT):
            Mp = ps_big.tile([P, C], f32, tag="big")
            for dt in range(CT):
                nc.tensor.matmul(Mp, lhsT=wv_sb[dt][:, it * P:(it + 1) * P],
                                 rhs=prT_sb[dt], start=(dt == 0), stop=(dt == CT - 1))
            MT_t = pmats.tile([P, C], bf, tag=f"MT{it}")
            nc.vector.tensor_copy(MT_t, Mp)
            MT_sb.append(MT_t)

        pvp = ps_big.tile([1, C], f32, tag="big")
        for dt in range(CT):
            nc.tensor.matmul(pvp, lhsT=bv_sb[dt], rhs=prT_sb[dt],
                             start=(dt == 0), stop=(dt == CT - 1))
        pvrow = consts.tile([1, C], bf, tag="pvrow")
        nc.vector.tensor_copy(pvrow, pvp)

        # ---------------- phase 8: attn^T + residual + store ----------------
        x_v = [x_sb[ci].rearrange("p (u f) -> p u f", u=N // SUB) for ci in range(CT)]
        for nt in range(NT):
            at = ps_big.tile([P, C], f32, tag="big")
            for it in range(CT):
                nc.tensor.matmul(at, lhsT=g_sb[it][:, nt * P:(nt + 1) * P],
                                 rhs=MT_sb[it], start=(it == 0), stop=False)
            nc.tensor.matmul(at, lhsT=ones1, rhs=pvrow, start=False, stop=True)

            resid = presid.tile([P, C], f32, tag="resid")
            ci, lo = nt // 8, nt % 8
            nc.scalar.dma_start(
                resid,
                x_sb[ci][16 * lo:16 * (lo + 1), :].rearrange(
                    "p (u f) -> p u f", u=8),
            )
            osb = pout.tile([P, C], f32, tag="o")
            nc.vector.tensor_tensor(osb, at, resid, OP.add)
            nc.sync.dma_start(out_d[nt * P:(nt + 1) * P, :], osb)

    nc.compile()
    return nc


_NC = None


def _get_program():
    global _NC
    if _NC is None:
        _NC = _build_program()
    return _NC


def _stage_inputs(x, gn_w, gn_b, wq, bq, wk, bk, wv, bv):
    """Build the per-core input maps (host-side sharding / layout prep)."""
    x = np.asarray(x, dtype=np.float32).reshape(B, C, N)
    shared = {
        "wqT": np.ascontiguousarray(np.asarray(wq, np.float32).T).astype(BF16),
        "wkT": np.ascontiguousarray(np.asarray(wk, np.float32).T).astype(BF16),
        "wv": np.ascontiguousarray(np.asarray(wv, np.float32)).astype(BF16),
        "bq_row": np.asarray(bq, np.float32).reshape(1, C).astype(BF16),
        "bk_row": np.asarray(bk, np.float32).reshape(1, C).astype(BF16),
        "bk_n": (float(N) * np.asarray(bk, np.float32)).reshape(1, C),
        "bv_col": np.asarray(bv, np.float32).reshape(C, 1).astype(BF16),
        "gnw": np.asarray(gn_w, np.float32).reshape(C, 1),
        "gnb": np.asarray(gn_b, np.float32).reshape(C, 1),
    }
    ind16 = np.zeros((C, 8), np.float32)
    indT = np.zeros((8, P), np.float32)
    for c in range(C):
        ind16[c, (c % P) // GS] = 1.0 / GS
    for p in range(P):
        indT[p // GS, p] = 1.0
    shared["ind16"] = ind16
    shared["indT01"] = indT

    in_maps = []
    for b in range(B):
        m = dict(shared)
        m["x"] = np.ascontiguousarray(x[b])
        in_maps.append(m)
    return in_maps


def kernel(x, gn_w, gn_b, wq, bq, wk, bk, wv, bv, _trace=False, _tmpdir=None):
    nc = _get_program()
    in_maps = _stage_inputs(x, gn_w, gn_b, wq, bq, wk, bk, wv, bv)
    res = bass_utils.run_bass_kernel_spmd(
        nc, in_maps, core_ids=list(range(B)), trace=_trace, tmpdir=_tmpdir,
    )
    out = np.stack([res.results[b]["out"].reshape(C, H, W) for b in range(B)])
    if _trace:
        kernel._last_results = res
    return out.astype(np.float32)


# revision 10
# speedup vs baseline: 1.1581x; 1.0442x over previous
"""ConvAttention (GroupNorm + channel attention + residual) on 8 Trainium2
NeuronCores, data-parallel over the batch dimension (B=8 -> 1 item/core).

Per-core algorithm (x is (C=512, N=4096) for one batch item):
  1. GroupNorm(32 groups) via per-channel bn_stats + tiny indicator matmuls
     for the cross-partition group reduction / broadcast; the affine
     (x*a + b) runs on the Scalar engine, producing g in bf16.
  2. scores = q k^T (contracted over N) is computed via the Gram matrix:
         scores = Wq (g g^T) Wk^T + (Wq sg) (x) bk + bq (x) (Wk sg + N bk)
     with sg = row sums of g.  g g^T needs g^T, produced by PE transposes
     that are pipelined with the Gram accumulation.
  3. softmax rows, fused: exp((s - max) * alpha) on the Scalar engine with
     accumulated row sums; probs scaled by 1/sum, then PE-transposed.
  4. attn^T = g^T M^T + 1 (x) (bv^T probs^T), with M^T = Wv^T probs^T.
     attn^T lands in (N, C) layout, which makes the reference's
     permute+reshape a flat-memory no-op:  out.flat = x.flat + attnT.flat.
  5. Residual x is re-streamed into the (N, C) tile layout by DMA and the
     final add is fused into the PSUM evacuation.
"""
import sys

if "/opt/trn_rl_repo" not in sys.path:
    sys.path.insert(0, "/opt/trn_rl_repo")

from contextlib import ExitStack

import ml_dtypes
import numpy as np

import concourse.bass as bass
import concourse.tile as tile
from concourse import bacc, mybir
from concourse import bass_utils
from concourse.masks import make_identity

BF16 = ml_dtypes.bfloat16
bf = mybir.dt.bfloat16
f32 = mybir.dt.float32

B, C, H, W = 8, 512, 64, 64
N = H * W            # 4096 spatial tokens
GROUPS = 32
GS = C // GROUPS     # 16 channels per group
EPS = 1e-6
ALPHA = float(C) ** -0.5
P = 128
CT = C // P          # 4 channel tiles
NT = N // P          # 32 spatial tiles
SUB = 512            # bn_stats subgroup width
NSUB = N // SUB      # 8

AF = mybir.ActivationFunctionType
AX = mybir.AxisListType
OP = mybir.AluOpType


def _build_program():
    nc = bacc.Bacc("TRN2", target_bir_lowering=False, debug=False, num_devices=B)

    x_d = nc.dram_tensor("x", (C, N), f32, kind="ExternalInput").ap()
    wqT_d = nc.dram_tensor("wqT", (C, C), bf, kind="ExternalInput").ap()
    wkT_d = nc.dram_tensor("wkT", (C, C), bf, kind="ExternalInput").ap()
    wv_d = nc.dram_tensor("wv", (C, C), bf, kind="ExternalInput").ap()
    bqr_d = nc.dram_tensor("bq_row", (1, C), bf, kind="ExternalInput").ap()
    bkr_d = nc.dram_tensor("bk_row", (1, C), bf, kind="ExternalInput").ap()
    bk4_d = nc.dram_tensor("bk_n", (1, C), f32, kind="ExternalInput").ap()
    bv_d = nc.dram_tensor("bv_col", (C, 1), bf, kind="ExternalInput").ap()
    gnw_d = nc.dram_tensor("gnw", (C, 1), f32, kind="ExternalInput").ap()
    gnb_d = nc.dram_tensor("gnb", (C, 1), f32, kind="ExternalInput").ap()
    i16_d = nc.dram_tensor("ind16", (C, 8), f32, kind="ExternalInput").ap()
    iT_d = nc.dram_tensor("indT01", (8, P), f32, kind="ExternalInput").ap()
    out_d = nc.dram_tensor("out", (N, C), f32, kind="ExternalOutput").ap()

    with tile.TileContext(nc) as tc, ExitStack() as ctx:
        consts = ctx.enter_context(tc.tile_pool(name="consts", bufs=1))
        px = ctx.enter_context(tc.tile_pool(name="px", bufs=1))
        pg = ctx.enter_context(tc.tile_pool(name="pg", bufs=1))
        pmats = ctx.enter_context(tc.tile_pool(name="pmats", bufs=1))
        pgt = ctx.enter_context(tc.tile_pool(name="pgt", bufs=3))
        psmall = ctx.enter_context(tc.tile_pool(name="psmall", bufs=4))
        presid = ctx.enter_context(tc.tile_pool(name="presid", bufs=6))
        pout = ctx.enter_context(tc.tile_pool(name="pout", bufs=6))
        # PSUM: 4 (gram accum) + 2 (transpose) + 2 (rotating matmul out) = 8 banks
        ps_big = ctx.enter_context(tc.tile_pool(name="ps_big", bufs=2, space="PSUM"))
        ps_ctx = ExitStack()
        ps_acc = ps_ctx.enter_context(tc.tile_pool(name="ps_acc", bufs=1, space="PSUM"))
        ps_tr = ps_ctx.enter_context(tc.tile_pool(name="ps_tr", bufs=2, space="PSUM"))

        # ---------------- constants / weights ----------------
        ident = consts.tile([P, P], bf, tag="ident")
        make_identity(nc, ident)

        wq_sb, wk_sb, wv_sb = [], [], []
        for t in range(CT):
            wq_t = consts.tile([P, C], bf, tag=f"wq{t}")
            nc.gpsimd.dma_start(wq_t, wqT_d[t * P:(t + 1) * P, :])
            wq_sb.append(wq_t)
            wk_t = consts.tile([P, C], bf, tag=f"wk{t}")
            nc.gpsimd.dma_start(wk_t, wkT_d[t * P:(t + 1) * P, :])
            wk_sb.append(wk_t)
            wv_t = consts.tile([P, C], bf, tag=f"wv{t}")
            nc.gpsimd.dma_start(wv_t, wv_d[t * P:(t + 1) * P, :])
            wv_sb.append(wv_t)

        bqr = consts.tile([1, C], bf, tag="bqr")
        nc.gpsimd.dma_start(bqr, bqr_d)
        bkr = consts.tile([1, C], bf, tag="bkr")
        nc.gpsimd.dma_start(bkr, bkr_d)
        bk4 = consts.tile([1, C], f32, tag="bk4")
        nc.gpsimd.dma_start(bk4, bk4_d)

        bv_sb, gnw_sb, gnb_sb, i16_sb = [], [], [], []
        for t in range(CT):
            bv_t = consts.tile([P, 1], bf, tag=f"bv{t}")
            nc.gpsimd.dma_start(bv_t, bv_d[t * P:(t + 1) * P, :])
            bv_sb.append(bv_t)
            gnw_t = consts.tile([P, 1], f32, tag=f"gnw{t}")
            nc.gpsimd.dma_start(gnw_t, gnw_d[t * P:(t + 1) * P, :])
            gnw_sb.append(gnw_t)
            gnb_t = consts.tile([P, 1], f32, tag=f"gnb{t}")
            nc.gpsimd.dma_start(gnb_t, gnb_d[t * P:(t + 1) * P, :])
            gnb_sb.append(gnb_t)
            i16_t = consts.tile([P, 8], f32, tag=f"i16{t}")
            nc.gpsimd.dma_start(i16_t, i16_d[t * P:(t + 1) * P, :])
            i16_sb.append(i16_t)
        iT_sb = consts.tile([8, P], f32, tag="iT")
        nc.gpsimd.dma_start(iT_sb, iT_d)
        eps8 = consts.tile([8, 1], f32, tag="eps8")
        nc.vector.memset(eps8, EPS)
        ones1 = consts.tile([1, P], bf, tag="ones1")
        nc.vector.memset(ones1, 1.0)

        # ---------------- phase 1+2: per-tile GroupNorm (fully pipelined) ------
        # Groups (16ch) never cross a 128-channel tile, so each tile reduces its
        # own 8 groups: stats -> local indicator matmuls -> affine, no barrier.
        dmae = [nc.sync, nc.scalar]  # alternate the two HWDGE queues
        x_sb, g_sb, sg_sb = [], [], []
        for ci in range(CT):
            x_t = px.tile([P, N], f32, tag=f"x{ci}")
            for q in range(4):
                eng = [nc.sync, nc.scalar][q % 2]
                sl = slice(q * N // 4, (q + 1) * N // 4)
                eng.dma_start(x_t[:, sl], x_d[ci * P:(ci + 1) * P, sl])
            x_sb.append(x_t)

            stats = psmall.tile([P, NSUB, 6], f32, tag="stats")
            xv = x_t.rearrange("p (s f) -> p s f", f=SUB)
            for s in range(NSUB):
                nc.vector.bn_stats(out=stats[:, s, :], in_=xv[:, s, :])
            mv = psmall.tile([P, 2], f32, tag=f"mv{ci}", bufs=1)
            nc.vector.bn_aggr(out=mv, in_=stats)

            # st2 = [mu_p, E[x^2]_p] per channel
            st2 = psmall.tile([P, 2], f32, tag="st2")
            nc.vector.tensor_copy(st2[:, 0:1], mv[:, 0:1])
            e2 = psmall.tile([P, 1], f32, tag="e2")
            nc.vector.tensor_scalar(e2, mv[:, 0:1], mv[:, 0:1], None, op0=OP.mult)
            nc.vector.tensor_tensor(st2[:, 1:2], e2, mv[:, 1:2], OP.add)
            # local 8-group reduction (1/16-weighted indicator)
            gst = ps_big.tile([8, 2], f32, tag="big")
            nc.tensor.matmul(gst, lhsT=i16_sb[ci], rhs=st2, start=True, stop=True)

            gtmp = psmall.tile([8, 1], f32, tag="gtmp")
            nc.vector.tensor_scalar(gtmp, gst[:, 0:1], gst[:, 0:1], None, op0=OP.mult)
            gvar = psmall.tile([8, 1], f32, tag="gvar")
            nc.vector.tensor_tensor(gvar, gst[:, 1:2], gtmp, OP.subtract)
            gsd = psmall.tile([8, 1], f32, tag="gsd")
            nc.scalar.activation(gsd, gvar, AF.Sqrt, bias=eps8, scale=1.0)
            grs = psmall.tile([8, 1], f32, tag="grs")
            nc.vector.reciprocal(grs, gsd)
            gr2 = psmall.tile([8, 2], f32, tag="gr2")
            nc.vector.tensor_copy(gr2[:, 0:1], gst[:, 0:1])
            nc.vector.tensor_copy(gr2[:, 1:2], grs)

            bc = ps_big.tile([P, 2], f32, tag="big")
            nc.tensor.matmul(bc, lhsT=iT_sb, rhs=gr2, start=True, stop=True)
            a_col = psmall.tile([P, 1], f32, tag=f"a{ci}", bufs=1)
            nc.vector.tensor_tensor(a_col, gnw_sb[ci], bc[:, 1:2], OP.mult)
            tmp = psmall.tile([P, 1], f32, tag="tmp")
            nc.vector.tensor_tensor(tmp, bc[:, 0:1], a_col, OP.mult)
            b_col = psmall.tile([P, 1], f32, tag=f"b{ci}", bufs=1)
            nc.vector.tensor_tensor(b_col, gnb_sb[ci], tmp, OP.subtract)

            g_t = pg.tile([P, N], bf, tag=f"g{ci}")
            if ci % 2 == 0:
                # Scalar engine: g = Identity(x * a + b)
                nc.scalar.activation(g_t, x_sb[ci], AF.Identity,
                                     bias=b_col, scale=a_col)
            else:
                # Vector engine: same affine, keeps both engines busy
                nc.vector.tensor_scalar(g_t, x_sb[ci], a_col, b_col,
                                        op0=OP.mult, op1=OP.add)
            g_sb.append(g_t)

            # sg = N * (a * mu_p + b)  (row sums of g), as bf16 column
            t2 = psmall.tile([P, 1], f32, tag="t2")
            nc.vector.tensor_tensor(t2, a_col, mv[:, 0:1], OP.mult)
            nc.vector.tensor_tensor(t2, t2, b_col, OP.add)
            sg_t = consts.tile([P, 1], bf, tag=f"sg{ci}")
            nc.vector.tensor_scalar(sg_t, t2, float(N), None, op0=OP.mult)
            sg_sb.append(sg_t)

        # ---------------- phase 3: Gram = g g^T, pipelined with g^T transposes ----------------
        G_ps = [ps_acc.tile([P, C], f32, tag=f"G{i}", name=f"Gps{i}")
                for i in range(CT)]
        # NOTE: tag G0 reused after gstats is fully consumed above.
        prev_gt = None
        for nt in range(NT + 1):
            if nt < NT:
                trp = ps_tr.tile([P, C], bf, tag="tr")
                for it in range(CT):
                    nc.tensor.transpose(trp[:, it * P:(it + 1) * P],
                                        g_sb[it][:, nt * P:(nt + 1) * P], ident)
                gt = pgt.tile([P, C], bf, tag="gt")
                nc.vector.tensor_copy(gt, trp)
            else:
                gt = None
            if prev_gt is not None:
                for io in range(CT):
                    nc.tensor.matmul(G_ps[io], lhsT=prev_gt[:, io * P:(io + 1) * P],
                                     rhs=prev_gt, start=(nt == 1), stop=(nt == NT))
            prev_gt = gt

        G_sb = []
        for io in range(CT):
            G_t = pmats.tile([P, C], bf, tag=f"Gm{io}")
            nc.vector.tensor_copy(G_t, G_ps[io])
            G_sb.append(G_t)

        # ---------------- phase 4: bias rows t1 = Wq sg, u = Wk sg + N bk ----------------
        t1p = ps_big.tile([1, C], f32, tag="big")
        for it in range(CT):
            nc.tensor.matmul(t1p, lhsT=sg_sb[it], rhs=wq_sb[it],
                             start=(it == 0), stop=(it == CT - 1))
        t2p = ps_big.tile([1, C], f32, tag="big")
        for it in range(CT):
            nc.tensor.matmul(t2p, lhsT=sg_sb[it], rhs=wk_sb[it],
                             start=(it == 0), stop=(it == CT - 1))
        t1row = consts.tile([1, C], bf, tag="t1row")
        nc.vector.tensor_copy(t1row, t1p)
        urow = consts.tile([1, C], bf, tag="urow")
        nc.vector.tensor_tensor(urow, t2p, bk4, OP.add)

        # ---------------- phase 5: A = Wq G, A^T ----------------
        A_sb = []
        for ct in range(CT):
            Ap = ps_big.tile([P, C], f32, tag="big")
            for it in range(CT):
                nc.tensor.matmul(Ap, lhsT=wq_sb[it][:, ct * P:(ct + 1) * P],
                                 rhs=G_sb[it], start=(it == 0), stop=(it == CT - 1))
            A_t = pmats.tile([P, C], bf, tag=f"A{ct}")
            nc.vector.tensor_copy(A_t, Ap)
            A_sb.append(A_t)

        AT_sb = [pmats.tile([P, C], bf, tag=f"AT{jt}", name=f"ATsb{jt}")
                 for jt in range(CT)]
        for ct in range(CT):
            trp = ps_tr.tile([P, C], bf, tag="tr")
            for jt in range(CT):
                nc.tensor.transpose(trp[:, jt * P:(jt + 1) * P],
                                    A_sb[ct][:, jt * P:(jt + 1) * P], ident)
            for jt in range(CT):
                nc.vector.tensor_copy(AT_sb[jt][:, ct * P:(ct + 1) * P],
                                      trp[:, jt * P:(jt + 1) * P])

        # ---------------- phase 6: scores + softmax ----------------
        pr_sb = []
        for ct in range(CT):
            scp = ps_big.tile([P, C], f32, tag="big")
            for jt in range(CT):
                nc.tensor.matmul(scp, lhsT=AT_sb[jt][:, ct * P:(ct + 1) * P],
                                 rhs=wk_sb[jt], start=(jt == 0), stop=False)
            nc.tensor.matmul(scp, lhsT=t1row[0:1, ct * P:(ct + 1) * P], rhs=bkr,
                             start=False, stop=False)
            nc.tensor.matmul(scp, lhsT=bqr[0:1, ct * P:(ct + 1) * P], rhs=urow,
                             start=False, stop=True)
            nm = psmall.tile([P, 1], f32, tag="nm")
            nc.vector.reduce_max(nm, scp, axis=AX.X, negate=True)
            nma = psmall.tile([P, 1], f32, tag="nma")
            nc.vector.tensor_scalar(nma, nm, ALPHA, None, op0=OP.mult)
            se = psmall.tile([P, 1], f32, tag="se")
            pr_t = pmats.tile([P, C], bf, tag=f"pr{ct}")
            nc.scalar.activation(pr_t, scp, AF.Exp, bias=nma, scale=ALPHA,
                                 accum_out=se)
            ri = psmall.tile([P, 1], f32, tag="ri")
            nc.vector.reciprocal(ri, se)
            nc.vector.tensor_scalar_mul(pr_t, pr_t, ri)
            pr_sb.append(pr_t)

        # probs^T
        prT_sb = [pmats.tile([P, C], bf, tag=f"prT{dt}", name=f"prTsb{dt}")
                  for dt in range(CT)]
        for ct in range(CT):
            trp = ps_tr.tile([P, C], bf, tag="tr")
            for dt in range(CT):
                nc.tensor.transpose(trp[:, dt * P:(dt + 1) * P],
                                    pr_sb[ct][:, dt * P:(dt + 1) * P], ident)
            for dt in range(CT):
                nc.vector.tensor_copy(prT_sb[dt][:, ct * P:(ct + 1) * P],
                                      trp[:, dt * P:(dt + 1) * P])

        # ---------------- phase 7: M^T = Wv^T probs^T, pv row ----------------
        MT_sb = []
        for it in range(CT):
            Mp = ps_big.tile([P, C], f32, tag="big")
            for dt in range(CT):
                nc.tensor.matmul(Mp, lhsT=wv_sb[dt][:, it * P:(it + 1) * P],
                                 rhs=prT_sb[dt], start=(dt == 0), stop=(dt == CT - 1))
            MT_t = pmats.tile([P, C], bf, tag=f"MT{it}")
            nc.vector.tensor_copy(MT_t, Mp)
            MT_sb.append(MT_t)

        pvp = ps_big.tile([1, C], f32, tag="big")
        for dt in range(CT):
            nc.tensor.matmul(pvp, lhsT=bv_sb[dt], rhs=prT_sb[dt],
                             start=(dt == 0), stop=(dt == CT - 1))
        pvrow = consts.tile([1, C], bf, tag="pvrow")
        nc.vector.tensor_copy(pvrow, pvp)

        # ------- phase 8: attn^T (n, c) + residual + store -------
        # attnT[n, c] = sum_i g[i, n] M[c, i] + pv[c]; flat (n, c) order equals
        # the reference's permute+reshape flat order, so out rows store
        # contiguously and the residual x streams in via a reshape DMA.
        ps_ctx.close()  # release gram/transpose banks
        ps_att = ctx.enter_context(tc.tile_pool(name="ps_att", bufs=4, space="PSUM"))
        for nt in range(NT):
            at = ps_att.tile([P, C], f32, tag="att", name=f"at{nt}")
            for it in range(CT):
                nc.tensor.matmul(at, lhsT=g_sb[it][:, nt * P:(nt + 1) * P],
                                 rhs=MT_sb[it], start=(it == 0), stop=False)
            nc.tensor.matmul(at, lhsT=ones1, rhs=pvrow, start=False, stop=True)

            resid = presid.tile([P, C], f32, tag="resid")
            ci, lo = nt // 8, nt % 8
            nc.scalar.dma_start(
                resid,
                x_sb[ci][16 * lo:16 * (lo + 1), :].rearrange(
                    "p (u f) -> p u f", u=8),
            )
            osb = pout.tile([P, C], f32, tag="o")
            nc.vector.tensor_tensor(osb, at, resid, OP.add)
            nc.sync.dma_start(out_d[nt * P:(nt + 1) * P, :], osb)

    nc.compile()
    return nc


_NC = None


def _get_program():
    global _NC
    if _NC is None:
        _NC = _build_program()
    return _NC


def _stage_inputs(x, gn_w, gn_b, wq, bq, wk, bk, wv, bv):
    """Build the per-core input maps (host-side sharding / layout prep)."""
    x = np.asarray(x, dtype=np.float32).reshape(B, C, N)
    shared = {
        "wqT": np.ascontiguousarray(np.asarray(wq, np.float32).T).astype(BF16),
        "wkT": np.ascontiguousarray(np.asarray(wk, np.float32).T).astype(BF16),
        "wv": np.ascontiguousarray(np.asarray(wv, np.float32)).astype(BF16),
        "bq_row": np.asarray(bq, np.float32).reshape(1, C).astype(BF16),
        "bk_row": np.asarray(bk, np.float32).reshape(1, C).astype(BF16),
        "bk_n": (float(N) * np.asarray(bk, np.float32)).reshape(1, C),
        "bv_col": np.asarray(bv, np.float32).reshape(C, 1).astype(BF16),
        "gnw": np.asarray(gn_w, np.float32).reshape(C, 1),
        "gnb": np.asarray(gn_b, np.float32).reshape(C, 1),
    }
    ind16 = np.zeros((C, 8), np.float32)
    indT = np.zeros((8, P), np.float32)
    for c in range(C):
        ind16[c, (c % P) // GS] = 1.0 / GS
    for p in range(P):
        indT[p // GS, p] = 1.0
    shared["ind16"] = ind16
    shared["indT01"] = indT

    in_maps = []
    for b in range(B):
        m = dict(shared)
        m["x"] = np.ascontiguousarray(x[b])
        in_maps.append(m)
    return in_maps


def kernel(x, gn_w, gn_b, wq, bq, wk, bk, wv, bv, _trace=False, _tmpdir=None):
    nc = _get_program()
    in_maps = _stage_inputs(x, gn_w, gn_b, wq, bq, wk, bk, wv, bv)
    res = bass_utils.run_bass_kernel_spmd(
        nc, in_maps, core_ids=list(range(B)), trace=_trace, tmpdir=_tmpdir,
    )
    out = np.stack([res.results[b]["out"].reshape(C, H, W) for b in range(B)])
    if _trace:
        kernel._last_results = res
    return out.astype(np.float32)
